# revision 4
# baseline (speedup 1.0000x reference)
"""Balanced CE loss + accuracy on 8 Trainium2 NeuronCores (Bass/Tile).

Reference computation (N = 16777216 elements):
    loss = -sum(where(t==1, 1.6*log(p), 0.4*log(1-p))) / N
    acc  = mean(round(p) == t)

Strategy (data-parallel over N, no collectives needed):
  Shard N across 8 cores; per core stream 2048-column sub-chunks
  ([128, 2048] tiles, 2 MB DMA each) so the DMA pipe never idles.
  Single-variable encoding z = p - t folds both classes into one value:
    t==1 -> z = p-1 in (-1,0),  y := 1-|z| = p
    t==0 -> z = p   in ( 0,1),  y := 1-|z| = 1-p
  so the per-element log term is ln(y) with class weight w = 1.2*t+0.4,
  and "correct" (round(p)==t) is exactly |z| < 0.5  <=>  ln(y) >= -ln2.
  Per sub-chunk:
    DVE : z = p - t            (tensor_tensor, int32 t converted on read)
    ACT : z <- Abs(z)          (in place; same act table as Ln)
    ACT : q = Ln(-z + 1) bf16  (fused accum -> S[s] = per-part sum ln(y))
    DVE : (q >= -ln2) @4x      (fused accum -> C[s] = correct count)
    DVE : (t >= 1) * q         (stt, fused accum -> S1[s] = sum_{t=1} ln p)
  Host folds the [128, 3*NSUB] partials in f64:
    loss = -(0.4*S + 1.2*S1)/N,  acc = C/N.
  Every engine stays under the ~44us DMA shadow (DVE ~40us, ACT ~30us),
  so the kernel runs at the per-core HBM streaming roofline.
"""

import sys

if "/opt/trn_rl_repo" not in sys.path:
    sys.path.insert(0, "/opt/trn_rl_repo")

import numpy as np

import concourse.bass as bass
import concourse.bacc as bacc
import concourse.tile as tile
from concourse import mybir
from concourse.bass_utils import run_bass_kernel_spmd

N_CORES = 8
N = 16777216
P = 128
SHARD = N // N_CORES          # 2097152 elements per core
COLS = SHARD // P             # 16384 columns per core
SUB = 2048                    # sub-chunk columns (8KB/partition DMA rows)
NSUB = COLS // SUB            # 8 sub-chunks

AF = mybir.ActivationFunctionType
OP = mybir.AluOpType
LN2 = 0.6931471805599453

_NC_CACHE = None


def build_bass():
    """Build the single-core Bass program (SPMD across 8 cores)."""
    global _NC_CACHE
    if _NC_CACHE is not None:
        return _NC_CACHE

    nc = bacc.Bacc("TRN2", target_bir_lowering=False, debug=False)

    p_in = nc.dram_tensor("p_in", [SHARD], mybir.dt.float32, kind="ExternalInput").ap()
    t_in = nc.dram_tensor("t_in", [SHARD], mybir.dt.int32, kind="ExternalInput").ap()
    # acc columns: [s] sum ln(y); [NSUB+s] sum_{t=1} ln(p); [2NSUB+s] count
    acc = nc.dram_tensor("acc", [P, 3 * NSUB], mybir.dt.float32, kind="ExternalOutput").ap()

    with tile.TileContext(nc) as tc:
        with (
            tc.tile_pool(name="io", bufs=5) as io_pool,
            tc.tile_pool(name="zp", bufs=3) as z_pool,
            tc.tile_pool(name="qp", bufs=3) as q_pool,
            tc.tile_pool(name="misc", bufs=1) as misc_pool,
        ):
            acc_sb = misc_pool.tile([P, 3 * NSUB], mybir.dt.float32, tag="acc")
            junk_b = misc_pool.tile([P, SUB], mybir.dt.bfloat16, tag="jb")
            junk_s = misc_pool.tile([P, SUB], mybir.dt.bfloat16, tag="js")

            for s in range(NSUB):
                off = s * SUB * P
                p_t = io_pool.tile([P, SUB], mybir.dt.float32, tag="p")
                t_t = io_pool.tile([P, SUB], mybir.dt.int32, tag="t")
                z_t = z_pool.tile([P, SUB], mybir.dt.float32, tag="z")
                q_t = q_pool.tile([P, SUB], mybir.dt.bfloat16, tag="q")
                # split the first sub-chunk's DMA/compute in half so the
                # pipeline starts earlier
                nhalf = 2 if s == 0 else 1
                hc = SUB // nhalf
                for h in range(nhalf):
                    ho = off + h * hc * P
                    hs = slice(h * hc, (h + 1) * hc)
                    nc.sync.dma_start(
                        p_t[:, hs], p_in[ho : ho + hc * P].rearrange("(p f) -> p f", p=P)
                    )
                    nc.sync.dma_start(
                        t_t[:, hs], t_in[ho : ho + hc * P].rearrange("(p f) -> p f", p=P)
                    )
                    # z = p - t
                    nc.vector.tensor_tensor(z_t[:, hs], p_t[:, hs], t_t[:, hs], OP.subtract)
                # z <- |z| (in place, same act table as Ln)
                nc.scalar.activation(z_t[:], z_t[:], AF.Abs)
                # q = ln(1 - |z|) = ln(y); accum -> S[s]
                nc.scalar.activation(q_t[:], z_t[:], AF.Ln, bias=1.0, scale=-1.0,
                                     accum_out=acc_sb[:, s : s + 1])
                # correct count: q >= -ln2  (bf16 in/out -> 4x DVE)
                nc.vector.tensor_scalar(junk_b[:], q_t[:], -LN2, None, OP.is_ge, OP.add,
                                        accum_out=acc_sb[:, 2 * NSUB + s : 2 * NSUB + s + 1])
                # S1[s] = sum_{t=1} q = sum_{t=1} ln(p)
                nc.vector.scalar_tensor_tensor(junk_s[:], t_t[:], 1, q_t[:],
                                               OP.is_ge, OP.mult,
                                               accum_out=acc_sb[:, NSUB + s : NSUB + s + 1])

            nc.sync.dma_start(acc[:], acc_sb[:])

    nc.finalize()
    _NC_CACHE = nc
    return nc


def make_in_maps(input, target):
    inp = np.ascontiguousarray(np.asarray(input, dtype=np.float32)).reshape(
        N_CORES, SHARD
    )
    tgt = np.ascontiguousarray(np.asarray(target, dtype=np.int32)).reshape(
        N_CORES, SHARD
    )
    return [{"p_in": inp[c], "t_in": tgt[c]} for c in range(N_CORES)]


def combine(results):
    """Host-side unshard: reduce the 8 cores' partial sums -> (loss, acc)."""
    S = S1 = C = 0.0
    for r in results:
        aa = np.asarray(r["acc"], dtype=np.float64)
        S += aa[:, 0:NSUB].sum()
        S1 += aa[:, NSUB : 2 * NSUB].sum()
        C += aa[:, 2 * NSUB : 3 * NSUB].sum()
    loss = -(0.4 * S + 1.2 * S1) / N
    acc = C / N
    return np.float32(loss), np.float32(acc)


def run_on_hw(input, target, **spmd_kwargs):
    nc = build_bass()
    in_maps = make_in_maps(input, target)
    return run_bass_kernel_spmd(nc, in_maps, list(range(N_CORES)), **spmd_kwargs)


def kernel(input, target):
    br = run_on_hw(input, target)
    return combine(br.results)
